# revision 54
# baseline (speedup 1.0000x reference)
"""Fused decoder block (LN->QKV->cache-merge attention->proj->LN->MLP) on 8
Trainium2 NeuronCores, data-parallel over the batch (2 rows/core).

v3: row-pipelined schedule. The two batch rows per core are independent, so
the ACT-bound attention of row 1 overlaps the PE-bound proj/LN2/FC1 of row 0.

Key ideas (cumulative):
- host-side cache compaction via update_mask: attention over [512 surviving
  cache keys ++ 512 new keys] = 1024 keys, no mask bias (softmax is
  permutation invariant).
- fp8e4 DoubleRow matmuls everywhere (QKV, scores, PV, proj, FC1, FC2);
  weights host-quantized at x64 scale; hi+lo residual passes for W1/W2/h2
  keep the MLP path accurate (rel err ~1.3e-2 < 2e-2).
- q/k swizzled into [32, 2, head, T] pair layout by SBUF->SBUF DMAs so the
  64-dim head contraction runs as DoubleRow (half cost).
- LN standardization only (gains/biases folded into weights host-side);
  sum-sq stats via fp8 DR on squared tiles; rstd/-mu broadcast across
  partitions by GPSIMD partition_broadcast (no PE, no PSUM).
- linear biases as extra fp8 contraction rows; softmax denominator from a
  ones column in V, reciprocal broadcast on Pool.
- x2 residual kept in bf16 to fit SBUF; PSUM pools sized to exactly 8 banks
  per phase.
"""

import numpy as np
import ml_dtypes

B, NP, N, C, H = 16, 512, 1024, 1024, 16
HD = C // H            # 64
HID = 4 * C            # 4096
EPS = 1e-5
NCORES = 8
RPC = B // NCORES      # batch rows per core
T = NP                 # queries per row
CT = C // 128          # feature tiles
NKC = N - NP           # surviving cache keys (512)
KTC = NKC // 128       # cache key tiles (4)
KTN = T // 128         # new key tiles (4)
KT = KTC + KTN         # total key tiles (8)
HPAIR = H // 2
NHT = HID // 128       # fc1 output chunks (32)
SCALE = HD ** -0.5
WS = 64.0              # weight quantization scale
OS = 16.0              # v / attention-output scale
VP = 80                # padded v row (64 d + 1 ones + 15 pad)
W2P = 1                # fc2 weight passes (1 = hi only, 2 = hi+lo residual)

_state = {}
fp8np = ml_dtypes.float8_e4m3


def _build_module(with_bias=True):
    import concourse.tile as tile
    from concourse import bacc, mybir

    f32 = mybir.dt.float32
    f32r = mybir.dt.float32r
    bf16 = mybir.dt.bfloat16
    fp8 = mybir.dt.float8e4
    AF = mybir.ActivationFunctionType
    OP = mybir.AluOpType
    DRm = mybir.MatmulPerfMode.DoubleRow

    nc = bacc.Bacc("TRN2", target_bir_lowering=False, debug=False)

    xT = nc.dram_tensor("xT", [RPC, 128, CT, T], f32r, kind="ExternalInput")
    kcD = nc.dram_tensor("kcD", [RPC, 128, 2, 4, NKC], fp8, kind="ExternalInput")
    kcP = nc.dram_tensor("kcP", [RPC, 128, 4, NKC], fp8, kind="ExternalInput")
    vcC = nc.dram_tensor("vcC", [RPC, 128, H * KTC * VP], fp8, kind="ExternalInput")
    wqk = nc.dram_tensor("wqk", [8, 128, 2, CT, 128], fp8, kind="ExternalInput")
    wv = nc.dram_tensor("wv", [4, 128, CT, 256], fp8, kind="ExternalInput")
    wp = nc.dram_tensor("wp", [CT, 128, CT, 128], fp8, kind="ExternalInput")
    w1 = nc.dram_tensor("w1", [NHT // 2, 128, 2, 2, CT, 128], fp8,
                        kind="ExternalInput")
    w2 = nc.dram_tensor("w2", [CT, 128, W2P, NHT, 128], fp8, kind="ExternalInput")
    if with_bias:
        wxp = nc.dram_tensor("wxp", [1, CT, 2, 128], fp8, kind="ExternalInput")
        wx1 = nc.dram_tensor("wx1", [1, NHT, 2, 128], fp8, kind="ExternalInput")
        wx2 = nc.dram_tensor("wx2", [1, CT, 2, 128], fp8, kind="ExternalInput")
    bqk = nc.dram_tensor("bqk", [128, 16], f32, kind="ExternalInput")
    vbias = nc.dram_tensor("vbias", [C], f32, kind="ExternalInput")
    ones = nc.dram_tensor("ones", [128, 1], f32r, kind="ExternalInput")
    outT = nc.dram_tensor("outT", [RPC, 128, CT, T], f32, kind="ExternalOutput")

    from contextlib import ExitStack
    with nc.allow_low_precision(reason="deliberate fp8/f32r staging; PSUM accumulation fp32"), \
         tile.TileContext(nc, pool_alloc_mode="queue") as tc, ExitStack() as es:
        # ---------- constants ----------
        consts = es.enter_context(tc.tile_pool(name="consts", bufs=1))
        ones_sb = consts.tile([128, 1], f32r)
        nc.sync.dma_start(ones_sb[:], ones.ap())
        ones_bf = consts.tile([128, 1], bf16)
        nc.vector.memset(ones_bf[:], 1.0)
        ones8 = consts.tile([128, 2, 16], fp8)
        nc.vector.memset(ones8[:], 1.0)
        bqk_sb = consts.tile([128, 16], f32)
        nc.sync.dma_start(bqk_sb[:], bqk.ap())
        if with_bias:
            vb_sb = consts.tile([128, C], bf16)
            nc.gpsimd.dma_start(vb_sb[:], vbias.ap()[None].to_broadcast((128, C)))
        if with_bias:
            wxp_sb = consts.tile([1, CT, 2, 128], fp8)
            nc.sync.dma_start(wxp_sb[:], wxp.ap())
            wx1_sb = consts.tile([1, NHT, 2, 128], fp8)
            nc.sync.dma_start(wx1_sb[:], wx1.ap())
            wx2_sb = consts.tile([1, CT, 2, 128], fp8)
            nc.sync.dma_start(wx2_sb[:], wx2.ap())
            xtr_p = consts.tile([1, 2, 512], fp8)
            nc.vector.memset(xtr_p[0:1, 0, :], OS)
            nc.vector.memset(xtr_p[0:1, 1, :], 0.0)
            xtr_1 = consts.tile([1, 2, 512], fp8)
            nc.vector.memset(xtr_1[0:1, 0, :], 1.0)
            nc.vector.memset(xtr_1[0:1, 1, :], 0.0)
        eps_sb = consts.tile([1, 1], f32)
        nc.vector.memset(eps_sb[:], EPS)

        def open_pool(nm, space=None):
            kw = dict(space=space) if space else {}
            cm = tc.tile_pool(name=nm, bufs=1, **kw)
            return cm, cm.__enter__()

        def close_pool(cm):
            cm.__exit__(None, None, None)

        # ---------- long-lived pools (ring-stack order) ----------
        cm_work, p_work = open_pool("p_work")
        cm_xs, p_xs = open_pool("p_xs")
        xs = [p_xs.tile([128, CT, T], f32r, tag=f"xs{r}", name=f"xs{r}")
              for r in range(RPC)]
        cm_oT, p_oT = open_pool("p_oT")
        oT = [p_oT.tile([128, CT, T], fp8, tag=f"oT{r}", name=f"oT{r}")
              for r in range(RPC)]
        cm_x2, p_x2 = open_pool("p_x2")
        x2s = [p_x2.tile([128, CT, T], bf16, tag=f"x2{r}", name=f"x2{r}")
               for r in range(RPC)]
        cm_g, p_g = open_pool("p_g")
        gs = [p_g.tile([128, NHT, T], fp8, tag=f"g{r}", name=f"g{r}")
              for r in range(RPC)]
        cm_h2, p_h2 = open_pool("p_h2")
        h2 = [p_h2.tile([128, 2, CT, T], fp8, tag=f"h2{r}", name=f"h2{r}")
              for r in range(RPC)]
        cm_kv, p_kv = open_pool("p_kv")
        kc_sb = [p_kv.tile([128, 2, 4, NKC], fp8, tag=f"kc{r}", name=f"kc{r}")
                 for r in range(RPC)]
        vkv = [p_kv.tile([128, 2, H, KTC, VP], fp8, tag=f"vkv{r}", name=f"vkv{r}")
               for r in range(RPC)]
        qD = [p_kv.tile([128, 2, 4, T], fp8, tag=f"qD{r}", name=f"qD{r}")
              for r in range(RPC)]
        kD = [p_kv.tile([128, 2, 4, T], fp8, tag=f"kD{r}", name=f"kD{r}")
              for r in range(RPC)]
        cm_att, apool = open_pool("p_att")
        cm_psatt, psatt = open_pool("ps_att", space="PSUM")
        cm_h1, p_h1 = open_pool("p_h1")
        h1 = [p_h1.tile([128, CT, T], fp8, tag=f"h1{r}", name=f"h1{r}")
              for r in range(RPC)]

        # ================= layernorm =================
        def layernorm(src, dst, stats_pool, st_tag, lhs_ones, src_cast,
                      dst_lo=None, stats_tag="misc"):
            s_t = stats_pool.tile([128, T], f32, tag=stats_tag,
                                  name=f"{st_tag}s", bufs=2)
            s_ps = s_t[0:1, :]
            for ct in range(CT):
                nc.tensor.matmul(s_ps, lhs_ones, src[:, ct, :],
                                 start=(ct == 0), stop=(ct == CT - 1))
            ss_t = stats_pool.tile([128, T], f32, tag=stats_tag,
                                   name=f"{st_tag}ss", bufs=2)
            ss_ps = ss_t[0:1, :]
            for c in range(CT // 2):
                sqs = p_work.tile([128, 2, T], fp8, tag="sqs", name="sqs", bufs=1)
                for j in range(2):
                    eng = nc.gpsimd if j else nc.vector
                    eng.tensor_mul(sqs[:, j, :], src_cast(src[:, 2 * c + j, :]),
                                   src_cast(src[:, 2 * c + j, :]))
                nc.tensor.matmul(ss_ps, ones8[:, :, 0:1], sqs[:],
                                 start=(c == 0), stop=(c == CT // 2 - 1),
                                 perf_mode=DRm)
            st = p_work.tile([97, T], f32, tag="st", name="st", bufs=1)
            negmu, msq, var, stdv = (st[0:1, :], st[32:33, :], st[64:65, :],
                                     st[96:97, :])
            nc.vector.tensor_scalar(negmu, s_ps, -1.0 / C, None, OP.mult)
            nc.vector.tensor_mul(msq, negmu, negmu)
            nc.vector.scalar_tensor_tensor(var, ss_ps, 1.0 / C, msq,
                                           OP.mult, OP.subtract)
            nc.scalar.activation(stdv, var, AF.Sqrt, bias=eps_sb[:])
            rstd = p_work.tile([1, T], f32, tag="rstd", name="rstd", bufs=2)
            nc.vector.reciprocal(rstd[:], stdv)
            A_sb = p_work.tile([128, T], f32, tag="Asb", name="Asb", bufs=1)
            nc.gpsimd.partition_broadcast(A_sb[:], rstd[:])
            M_sb = p_work.tile([128, T], f32, tag="Msb", name="Msb", bufs=1)
            nc.gpsimd.partition_broadcast(M_sb[:], negmu)
            for ct in range(CT):
                tmp = p_work.tile([128, T], f32, tag="tmp", name="tmp", bufs=2)
                e1, e2 = ((nc.gpsimd, nc.vector) if ct % 3 == 2
                          else (nc.vector, nc.gpsimd)
                          if ct % 3 == 1 else (nc.vector, nc.vector))
                e1.tensor_add(tmp[:], src_cast(src[:, ct, :]), M_sb[:])
                if dst_lo is None:
                    e2.tensor_mul(dst[:, ct, :], tmp[:], A_sb[:])
                else:
                    e2.tensor_mul(tmp[:], tmp[:], A_sb[:])
                    e1.tensor_copy(dst[:, ct, :], tmp[:])
                    nc.vector.scalar_tensor_tensor(
                        dst_lo[:, ct, :], dst[:, ct, :], -1.0, tmp[:],
                        OP.mult, OP.add)

        # ================= emitters =================
        def emit_qk_pair(f, r, wpool, mmps, qsb, ksb):
            wt = wpool.tile([128, 2, CT, 128], fp8, tag="wqk", name="wqk", bufs=3)
            nc.sync.dma_start(wt[:], wqk.ap()[f])
            for which in range(2):
                ps = mmps.tile([128, T], f32, tag="mm", name="mm", bufs=2)
                for c in range(CT // 2):
                    nc.tensor.matmul(ps[:], wt[:, which, 2 * c:2 * c + 2, :],
                                     h1[r][:, 2 * c:2 * c + 2, :],
                                     start=(c == 0), stop=(c == CT // 2 - 1),
                                     perf_mode=DRm)
                dst = (qsb if which == 0 else ksb)[:, f, :]
                nc.vector.tensor_scalar(dst, ps[:], 1.0 / WS,
                                        bqk_sb[:, 8 * which + f:8 * which + f + 1],
                                        OP.mult, OP.add)

        def emit_v_chunk(ch, r, wpool, mmps):
            wvt = wpool.tile([128, CT, 256], fp8, tag="wv", name="wv", bufs=2)
            nc.sync.dma_start(wvt[:], wv.ap()[ch])
            for tt in range(KTN):
                psf = mmps.tile([128, T], f32, tag="mm", name="mm", bufs=2)
                ps = psf[:, 0:256]
                for c in range(CT // 2):
                    nc.tensor.matmul(
                        ps, h1[r][:, 2 * c:2 * c + 2, tt * 128:(tt + 1) * 128],
                        wvt[:, 2 * c:2 * c + 2, :],
                        start=(c == 0), stop=(c == CT // 2 - 1), perf_mode=DRm)
                if with_bias:
                    nc.vector.scalar_tensor_tensor(
                        vkv[r][:, 1, 4 * ch:4 * ch + 4, tt, 0:HD],
                        ps.rearrange("p (h d) -> p h d", h=4), OS / WS,
                        vb_sb[:, ch * 256:(ch + 1) * 256]
                        .rearrange("p (h d) -> p h d", h=4),
                        OP.mult, OP.add)
                else:
                    nc.vector.tensor_scalar(
                        vkv[r][:, 1, 4 * ch:4 * ch + 4, tt, 0:HD],
                        ps.rearrange("p (h d) -> p h d", h=4), OS / WS,
                        None, OP.mult)

        def emit_swizzle(r, qsb, ksb, fq):
            # head h=2f+par -> block d=2*par+f//4, hg=f%4
            for src_sb, dst in ((qsb, qD[r]), (ksb, kD[r])):
                for par in range(2):
                    for half in range(2):
                        nc.sync.dma_start(
                            dst[32 * (2 * par + fq):32 * (2 * par + fq) + 32,
                                half, :, :],
                            src_sb[64 * par + 32 * half:
                                   64 * par + 32 * half + 32,
                                   4 * fq:4 * fq + 4, :])

        def emit_attention(hp, r):
            for hh in range(2):
                h = 2 * hp + hh
                d = 2 * (h % 2) + (h // 2) // 4
                hg = (h // 2) % 4
                pb = 32 * d
                pt = apool.tile([128, KT, T], fp8, tag="pt", name="pt", bufs=2)
                pv = psatt.tile([128, T], f32, tag="pv", name="pv", bufs=2)
                for g in range(KT // 2):
                    sc = psatt.tile([128, 2, T], f32, tag="sc", name="sc", bufs=2)
                    for j in range(2):
                        kt = 2 * g + j
                        if kt < KTC:
                            lhs = kc_sb[r][pb:pb + 32, :, hg,
                                           kt * 128:(kt + 1) * 128]
                        else:
                            lhs = kD[r][pb:pb + 32, :, hg,
                                        (kt - KTC) * 128:(kt - KTC + 1) * 128]
                        nc.tensor.matmul(sc[:, j, :], lhs,
                                         qD[r][pb:pb + 32, :, hg, :],
                                         start=True, stop=True, perf_mode=DRm,
                                         tile_position=(pb, 0))
                    nc.scalar.activation(
                        pt[:, 2 * g:2 * g + 2, :].rearrange("p a b -> p (a b)"),
                        sc[:].rearrange("p a b -> p (a b)"), AF.Exp, scale=SCALE)
                    half = 0 if g < KTC // 2 else 1
                    koff = 0 if g < KTC // 2 else KTC
                    nc.tensor.matmul(
                        pv[0:VP, :],
                        vkv[r][:, half, h, 2 * g - koff:2 * g - koff + 2, :],
                        pt[:, 2 * g:2 * g + 2, :],
                        start=(g == 0), stop=(g == KT // 2 - 1), perf_mode=DRm)
                rd = apool.tile([1, T], f32, tag="rd", name="rd", bufs=2)
                nc.vector.reciprocal(rd[:], pv[HD:HD + 1, :])
                bcs = apool.tile([HD, T], f32, tag="bcs", name="bcs", bufs=2)
                nc.gpsimd.partition_broadcast(bcs[:], rd[:])
                nc.vector.tensor_mul(oT[r][64 * hh:64 * hh + 64, hp, :],
                                     pv[0:HD, :], bcs[:])

        def emit_attention_plain(hp, r, qsb, ksb, kcp):
            # scores as single 64-partition plain fp8 matmuls (hd halves are
            # partition-contiguous in the staging layout) -- no swizzle dep.
            for hh in range(2):
                h = 2 * hp + hh
                pt = apool.tile([128, KT, T], fp8, tag="pt", name="pt", bufs=2)
                pv = psatt.tile([128, T], f32, tag="pv", name="pv", bufs=2)
                for g in range(KT // 2):
                    sc = psatt.tile([128, 2, T], f32, tag="sc", name="sc", bufs=2)
                    for j in range(2):
                        kt = 2 * g + j
                        if kt < KTC:
                            lhs = kcp[64 * hh:64 * hh + 64, hp,
                                      kt * 128:(kt + 1) * 128]
                        else:
                            lhs = ksb[64 * hh:64 * hh + 64, hp,
                                      (kt - KTC) * 128:(kt - KTC + 1) * 128]
                        nc.tensor.matmul(sc[:, j, :], lhs,
                                         qsb[64 * hh:64 * hh + 64, hp, :],
                                         start=True, stop=True,
                                         tile_position=(64 * hh, 0))
                    nc.scalar.activation(
                        pt[:, 2 * g:2 * g + 2, :].rearrange("p a b -> p (a b)"),
                        sc[:].rearrange("p a b -> p (a b)"), AF.Exp, scale=SCALE)
                    half2 = 0 if g < KTC // 2 else 1
                    koff = 0 if g < KTC // 2 else KTC
                    nc.tensor.matmul(
                        pv[0:VP, :],
                        vkv[r][:, half2, h, 2 * g - koff:2 * g - koff + 2, :],
                        pt[:, 2 * g:2 * g + 2, :],
                        start=(g == 0), stop=(g == KT // 2 - 1), perf_mode=DRm)
                rd = apool.tile([1, T], f32, tag="rd", name="rd", bufs=2)
                nc.vector.reciprocal(rd[:], pv[HD:HD + 1, :])
                bcs = apool.tile([HD, T], f32, tag="bcs", name="bcs", bufs=2)
                nc.gpsimd.partition_broadcast(bcs[:], rd[:])
                nc.vector.tensor_mul(oT[r][64 * hh:64 * hh + 64, hp, :],
                                     pv[0:HD, :], bcs[:])

        def emit_proj(co, r, wpool, mpool):
            wt = wpool.tile([128, CT, 128], fp8, tag="wp", name="wp", bufs=3)
            nc.sync.dma_start(wt[:], wp.ap()[co])
            ps = mpool.tile([128, T], f32, tag="misc", name="misc", bufs=2)
            for c in range(CT // 2):
                nc.tensor.matmul(ps[:], wt[:, 2 * c:2 * c + 2, :],
                                 oT[r][:, 2 * c:2 * c + 2, :],
                                 start=(c == 0),
                                 stop=(not with_bias and c == CT // 2 - 1),
                                 perf_mode=DRm)
            if with_bias:
                nc.tensor.matmul(ps[:], wxp_sb[0:1, co, :, :], xtr_p[0:1, :, :],
                                 start=False, stop=True, perf_mode=DRm)
            nc.vector.scalar_tensor_tensor(
                x2s[r][:, co, :], ps[:], 1.0 / (OS * WS),
                xs[r][:, co, :].bitcast(f32), OP.mult, OP.add)

        def emit_fc1_pair(htp, rows, wpool, mpool):
            wt = wpool.tile([128, 2, 2, CT, 128], fp8, tag="w1", name="w1", bufs=3)
            nc.sync.dma_start(wt[:], w1.ap()[htp])
            for r in rows:
                for j in range(2):
                    ht = 2 * htp + j
                    ps = mpool.tile([128, T], f32, tag="misc", name="misc", bufs=2)
                    for pi, (pw, ph) in enumerate(((0, 0), (0, 1), (1, 0))):
                        for c in range(CT // 2):
                            nc.tensor.matmul(ps[:],
                                             wt[:, j, pw, 2 * c:2 * c + 2, :],
                                             h2[r][:, ph, 2 * c:2 * c + 2, :],
                                             start=(pi == 0 and c == 0),
                                             stop=(not with_bias and pi == 2
                                                   and c == CT // 2 - 1),
                                             perf_mode=DRm)
                    if with_bias:
                        nc.tensor.matmul(ps[:], wx1_sb[0:1, ht, :, :],
                                         xtr_1[0:1, :, :],
                                         start=False, stop=True, perf_mode=DRm)
                    nc.scalar.activation(gs[r][:, ht, :], ps[:], AF.Gelu,
                                         scale=1.0 / WS)

        def emit_fc2(co, r, wt, f2pool, opool):
            ps = f2pool.tile([128, T], f32, tag="fc2", name="fc2", bufs=3)
            for pw in range(W2P):
                for tp in range(NHT // 2):
                    nc.tensor.matmul(ps[:], wt[:, pw, 2 * tp:2 * tp + 2, :],
                                     gs[r][:, 2 * tp:2 * tp + 2, :],
                                     start=(pw == 0 and tp == 0),
                                     stop=(not with_bias and pw == W2P - 1
                                           and tp == NHT // 2 - 1),
                                     perf_mode=DRm)
            if with_bias:
                nc.tensor.matmul(ps[:], wx2_sb[0:1, co, :, :], xtr_1[0:1, :, :],
                                 start=False, stop=True, perf_mode=DRm)
            ot = opool.tile([128, T], f32, tag="ot", name="ot", bufs=3)
            nc.vector.scalar_tensor_tensor(
                ot[:], ps[:], 1.0 / WS, x2s[r][:, co, :], OP.mult, OP.add)
            nc.sync.dma_start(outT.ap()[r, :, co, :], ot[:])

        # ================= phase 0: loads + LN1 =================
        cm_psln, psln = open_pool("ps_ln", space="PSUM")
        for r in range(RPC):
            for qr in range(4):
                nc.sync.dma_start(
                    xs[r][:, 2 * qr:2 * qr + 2, :],
                    xT.ap()[r, :, 2 * qr:2 * qr + 2, :])
            nc.sync.dma_start(
                kc_sb[r][:].rearrange("p a b c -> p (a b c)"), kcD.ap()[r])
            nc.sync.dma_start(
                vkv[r][:, 0, :, :, :].rearrange("p h k d -> p (h k d)"),
                vcC.ap()[r])
            nc.gpsimd.memset(vkv[r][:, 1, :, :, HD:HD + 1], 1.0)
            nc.gpsimd.memset(vkv[r][:, 1, :, :, HD + 1:VP], 0.0)
        layernorm(xs[0], h1[0], psln, "l1", ones_sb[:, 0:1],
                  lambda ap: ap.bitcast(f32))
        close_pool(cm_psln)

        # ================= qkv r0 + swizzle r0 (warmup-critical) =========
        cm_wA, wA = open_pool("p_wA")
        cm_psmm, psmm = open_pool("ps_mm", space="PSUM")
        cm_st0, p_st0 = open_pool("p_st0")
        q0 = p_st0.tile([128, CT, T], fp8, tag="q0", name="q0")
        k0 = p_st0.tile([128, CT, T], fp8, tag="k0", name="k0")
        kcp0 = p_st0.tile([128, 4, NKC], fp8, tag="kcp0", name="kcp0")
        nc.sync.dma_start(kcp0[:], kcP.ap()[0])
        emit_qk_pair(0, 0, wA, psmm, q0, k0)
        emit_qk_pair(1, 0, wA, psmm, q0, k0)
        emit_v_chunk(0, 0, wA, psmm)
        emit_attention_plain(0, 0, q0, k0, kcp0)
        emit_attention_plain(1, 0, q0, k0, kcp0)
        emit_qk_pair(2, 0, wA, psmm, q0, k0)
        emit_qk_pair(3, 0, wA, psmm, q0, k0)
        emit_v_chunk(1, 0, wA, psmm)
        emit_attention_plain(2, 0, q0, k0, kcp0)
        emit_attention_plain(3, 0, q0, k0, kcp0)
        for f in range(4, 8):
            emit_qk_pair(f, 0, wA, psmm, q0, k0)
        emit_swizzle(0, q0, k0, 1)
        # r1's LN1 is not needed until phase A -- emit after r0's warmup chain
        layernorm(xs[1], h1[1], psmm, "l1b", ones_sb[:, 0:1],
                  lambda ap: ap.bitcast(f32), stats_tag="mm")
        emit_v_chunk(2, 0, wA, psmm)
        emit_v_chunk(3, 0, wA, psmm)
        close_pool(cm_st0)

        # ================= phase A: attention r0 || qkv r1 =================
        cm_st1, p_st1 = open_pool("p_st1")
        q1 = p_st1.tile([128, CT, T], fp8, tag="q1", name="q1")
        k1 = p_st1.tile([128, CT, T], fp8, tag="k1", name="k1")
        for hp in range(4, HPAIR):
            emit_attention(hp, 0)
        for f in range(8):
            emit_qk_pair(f, 1, wA, psmm, q1, k1)
            if f % 2 == 1:
                emit_v_chunk(f // 2, 1, wA, psmm)
            if f == 3:
                emit_swizzle(1, q1, k1, 0)
        emit_swizzle(1, q1, k1, 1)
        close_pool(cm_st1)
        close_pool(cm_psmm)
        close_pool(cm_wA)
        close_pool(cm_h1)

        # ====== phase B: attention r1 (priority) || proj/LN2/FC1 r0 ======
        cm_psB, psB = open_pool("ps_B", space="PSUM")
        cm_wB, wB = open_pool("p_wB")
        for hp in range(HPAIR):
            emit_attention(hp, 1)
        for co in range(CT):
            emit_proj(co, 0, wB, psB)
        layernorm(x2s[0], h2[0][:, 0], psB, "l2a", ones_bf[:],
                  lambda ap: ap, dst_lo=h2[0][:, 1])
        for htp in range(NHT // 2):
            emit_fc1_pair(htp, [0], wB, psB)
        close_pool(cm_wB)
        close_pool(cm_psB)
        close_pool(cm_psatt)
        close_pool(cm_att)
        close_pool(cm_kv)

        # ================= tail =================
        with tc.tile_pool(name="ps_T", space="PSUM", bufs=1) as psT, \
             tc.tile_pool(name="p_wT", bufs=1) as wT, \
             tc.tile_pool(name="p_osb", bufs=1) as osb:
            for co in range(CT):
                emit_proj(co, 1, wT, psT)
            layernorm(x2s[1], h2[1][:, 0], psT, "l2b", ones_bf[:],
                      lambda ap: ap, dst_lo=h2[1][:, 1])
            for co in range(CT):
                w2t = wT.tile([128, W2P, NHT, 128], fp8, tag="w2", name="w2", bufs=2)
                nc.sync.dma_start(w2t[:], w2.ap()[co])
                emit_fc2(co, 0, w2t, psT, osb)
            for htp in range(NHT // 2):
                emit_fc1_pair(htp, [1], wT, psT)
            for co in range(CT):
                w2t = wT.tile([128, W2P, NHT, 128], fp8, tag="w2", name="w2", bufs=2)
                nc.sync.dma_start(w2t[:], w2.ap()[co])
                emit_fc2(co, 1, w2t, psT, osb)
        close_pool(cm_h2)
        close_pool(cm_g)
        close_pool(cm_x2)
        close_pool(cm_oT)
        close_pool(cm_xs)
        close_pool(cm_work)

    nc.compile()
    return nc


class _Runner:
    """Hold the compiled PJRT executable (mirrors bass2jax.run_bass_via_pjrt)."""

    def __init__(self, nc, n_cores):
        import jax
        from jax.sharding import Mesh, PartitionSpec
        from jax.experimental.shard_map import shard_map
        import concourse.mybir as mybir
        from concourse.bass2jax import (
            install_neuronx_cc_hook, partition_id_tensor, _bass_exec_p)

        install_neuronx_cc_hook()
        self.jax = jax
        self.n_cores = n_cores
        partition_name = nc.partition_id_tensor.name if nc.partition_id_tensor else None
        in_names, out_names, out_avals, zero_outs = [], [], [], []
        for alloc in nc.m.functions[0].allocations:
            if not isinstance(alloc, mybir.MemoryLocationSet):
                continue
            name = alloc.memorylocations[0].name
            if alloc.kind == "ExternalInput":
                if name != partition_name:
                    in_names.append(name)
            elif alloc.kind == "ExternalOutput":
                shape = tuple(alloc.tensor_shape)
                dtype = mybir.dt.np(alloc.dtype)
                out_names.append(name)
                out_avals.append(jax.core.ShapedArray(shape, dtype))
                zero_outs.append(np.zeros(shape, dtype))
        self.in_names, self.out_names = in_names, out_names
        self.out_avals, self.zero_outs = out_avals, zero_outs
        self.n_params = len(in_names)
        all_names = in_names + out_names
        if partition_name is not None:
            all_names.append(partition_name)

        def _body(*args):
            operands = list(args)
            if partition_name is not None:
                operands.append(partition_id_tensor())
            return tuple(
                _bass_exec_p.bind(
                    *operands,
                    out_avals=tuple(out_avals),
                    in_names=tuple(all_names),
                    out_names=tuple(out_names),
                    lowering_input_output_aliases=(),
                    sim_require_finite=True,
                    sim_require_nnan=True,
                    nc=nc,
                ))

        devices = jax.devices()[:n_cores]
        assert len(devices) == n_cores, f"need {n_cores} cores, have {len(jax.devices())}"
        mesh = Mesh(np.asarray(devices), ("core",))
        n_outs = len(out_names)
        self._fn = jax.jit(
            shard_map(_body, mesh=mesh,
                      in_specs=(PartitionSpec("core"),) * (self.n_params + n_outs),
                      out_specs=(PartitionSpec("core"),) * n_outs,
                      check_rep=False),
            keep_unused=True)

    def prepare(self, in_maps):
        np_ = np
        per_core = [[np_.asarray(m[n]) for n in self.in_names] for m in in_maps]
        concat_in = [
            np_.concatenate([per_core[c][i] for c in range(self.n_cores)], axis=0)
            for i in range(self.n_params)]
        concat_zeros = [
            np_.zeros((self.n_cores * z.shape[0], *z.shape[1:]), z.dtype)
            for z in self.zero_outs]
        return self.jax.device_put(concat_in + concat_zeros)

    def run(self, prepared):
        out = self._fn(*prepared)
        self.jax.block_until_ready(out)
        return out

    def results(self, out_arrs):
        return [
            {name: np.asarray(out_arrs[i]).reshape(
                self.n_cores, *self.out_avals[i].shape)[c]
             for i, name in enumerate(self.out_names)}
            for c in range(self.n_cores)]


def _get_runner(with_bias):
    if "runner" not in _state:
        nc = _build_module(with_bias=with_bias)
        _state["nc"] = nc
        _state["runner"] = _Runner(nc, NCORES)
    return _state["runner"]


def _prepare_in_maps(x, cache_k, cache_v, update_mask, qkv_w, qkv_b, proj_w,
                     proj_b, n1_g, n1_b, n2_g, n2_b, fc1_w, fc1_b, fc2_w, fc2_b):
    f32 = np.float32
    x = np.asarray(x, f32)
    cache_k = np.asarray(cache_k, f32)
    cache_v = np.asarray(cache_v, f32)
    update_mask = np.asarray(update_mask, bool)
    qkv_w = np.asarray(qkv_w, f32)
    qkv_b = np.asarray(qkv_b, f32)
    proj_w = np.asarray(proj_w, f32)
    proj_b = np.asarray(proj_b, f32)
    n1_g = np.asarray(n1_g, f32)
    n1_b = np.asarray(n1_b, f32)
    n2_g = np.asarray(n2_g, f32)
    n2_b = np.asarray(n2_b, f32)
    fc1_w = np.asarray(fc1_w, f32)
    fc1_b = np.asarray(fc1_b, f32)
    fc2_w = np.asarray(fc2_w, f32)
    fc2_b = np.asarray(fc2_b, f32)

    xT = np.ascontiguousarray(
        x.transpose(0, 2, 1).reshape(B, CT, 128, T).transpose(0, 2, 1, 3))

    kcD = np.empty((B, 128, 2, 4, NKC), fp8np)
    kcP = np.empty((B, 128, 4, NKC), fp8np)
    vcC = np.empty((B, 128, H, KTC, VP), fp8np)
    for b in range(B):
        keep = ~update_mask[b]
        kc = cache_k[b][:, keep, :]          # [H, NKC, HD]
        vc = cache_v[b][:, keep, :]
        kt_ = kc.transpose(0, 2, 1)          # [H, HD, NKC]
        for h in range(8):                   # plain layout for hp0-hp3
            kcP[b, 64 * (h % 2):64 * (h % 2) + 64, h // 2, :] = (
                kt_[h].astype(fp8np))
        for h in range(H):
            d_ = 2 * (h % 2) + (h // 2) // 4
            hg_ = (h // 2) % 4
            for half in range(2):
                kcD[b, 32 * d_:32 * d_ + 32, half, hg_, :] = (
                    kt_[h, 32 * half:32 * half + 32, :].astype(fp8np))
        vv = (vc.transpose(1, 0, 2).reshape(KTC, 128, H, HD)
              .transpose(1, 2, 0, 3))        # [128, H, KTC, HD]
        pad = np.zeros((128, H, KTC, VP - HD), f32)
        pad[:, :, :, 0] = 1.0
        vcC[b] = np.concatenate([vv * OS, pad], axis=3).astype(fp8np)

    def wtile(w, nf, cols):
        ci = w.shape[0]
        return np.ascontiguousarray(
            (WS * w).reshape(ci // 128, 128, nf, cols)
            .transpose(2, 1, 0, 3)).astype(fp8np)

    def wtile_hl(w, nf, cols):
        ws = WS * w
        hi = ws.astype(fp8np)
        lo = (ws - hi.astype(np.float32)).astype(fp8np)
        ci = w.shape[0]

        def t(a):
            return (a.reshape(ci // 128, 128, nf, cols).transpose(2, 1, 0, 3))
        return np.ascontiguousarray(
            np.stack([t(hi), t(lo)], axis=2)).astype(fp8np)

    wqkv_eff = n1_g[:, None] * qkv_w
    bias_qkv = n1_b @ qkv_w + qkv_b
    wqk16 = wtile(wqkv_eff[:, :2048], 16, 128)       # [16,128,CT,128]
    wqk_t = np.empty((8, 128, 2, CT, 128), fp8np)
    for f in range(8):
        wqk_t[f, :, 0] = wqk16[f]
        wqk_t[f, :, 1] = wqk16[8 + f]
    wv_t = wtile(wqkv_eff[:, 2048:], 4, 256)
    bqk_t = np.ascontiguousarray(bias_qkv[:2048].reshape(16, 128).T).astype(f32)
    vbias_t = (OS * bias_qkv[2048:]).astype(f32)

    wp_t = wtile(proj_w, CT, 128)
    wxp_t = np.zeros((1, CT, 2, 128), fp8np)
    wxp_t[0, :, 0, :] = (WS * proj_b).reshape(CT, 128).astype(fp8np)

    w1_eff = n2_g[:, None] * fc1_w
    bias_fc1 = n2_b @ fc1_w + fc1_b
    w1_hl = wtile_hl(w1_eff, NHT, 128)               # [32,128,2,CT,128]
    w1_t = np.ascontiguousarray(
        w1_hl.reshape(NHT // 2, 2, 128, 2, CT, 128)
        .transpose(0, 2, 1, 3, 4, 5))                # [16,128,2,2,CT,128]
    wx1_t = np.zeros((1, NHT, 2, 128), fp8np)
    wx1_t[0, :, 0, :] = (WS * bias_fc1).reshape(NHT, 128).astype(fp8np)

    w2_t = np.ascontiguousarray(
        wtile_hl(fc2_w, CT, 128)[:, :, :W2P])        # [CT,128,W2P,NHT,128]
    wx2_t = np.zeros((1, CT, 2, 128), fp8np)
    wx2_t[0, :, 0, :] = (WS * fc2_b).reshape(CT, 128).astype(fp8np)

    with_bias = bool(np.any(wxp_t) or np.any(wx1_t) or np.any(wx2_t))
    shared = dict(
        wqk=wqk_t, wv=wv_t, wp=wp_t, w1=w1_t, w2=w2_t,
        bqk=bqk_t, vbias=vbias_t,
        ones=np.ones((128, 1), f32),
    )
    if with_bias:
        shared.update(wxp=wxp_t, wx1=wx1_t, wx2=wx2_t)
    in_maps = []
    for c in range(NCORES):
        s = slice(c * RPC, (c + 1) * RPC)
        in_maps.append(dict(
            shared, xT=xT[s], kcD=kcD[s], kcP=kcP[s],
            vcC=vcC[s].reshape(RPC, 128, H * KTC * VP)))
    return in_maps, with_bias


def kernel(**inputs) -> np.ndarray:
    in_maps, with_bias = _prepare_in_maps(**inputs)
    runner = _get_runner(with_bias)
    prepared = runner.prepare(in_maps)
    out = runner.run(prepared)
    res = runner.results(out)
    full = np.empty((B, NP, C), np.float32)
    for c in range(NCORES):
        for r in range(RPC):
            full[c * RPC + r] = res[c]["outT"][r].transpose(2, 1, 0).reshape(T, C)
    return full
